# revision 1
# baseline (speedup 1.0000x reference)
"""Trainium2 Bass kernel for Llama GQA attention (B=2, S=2048, H=4096,
32 Q heads / 8 KV heads, head_dim 128, RoPE, causal).

Sharding: tensor-parallel by head across 8 cores. Core c owns Q heads
[4c..4c+3] and KV head c. Each core computes its Q/K/V projections,
RoPE, causal attention, and a partial output projection over its 512
attention features; the host sums the 8 partial outputs.

Device layout is feature-major ([feature, token]) throughout:
  - QKV proj:  Q'[f,t] (psum) = sum_h WqT[h,f].T @ xT[h,t]     (bf16)
  - RoPE:      q*cos + swap_halves(q)*sign*sin  (DVE + DMA swap)
  - scores:    S.T[k,q] = K'[d,k].T @ Q'[d,q]   (softmax over partition)
  - softmax:   exp on ACT (no max subtraction; scores are O(10)),
               denominator via ones-column matmul, fast reciprocal,
               K=1 broadcast matmul, normalize fused into psum evict
  - AV:        U[d,q] = Vtok[k,d].T @ E[k,q]    (bf16, causal-sliced)
  - out:       out[t,o] = attn'[f,t].T @ WoT[f,o]  (partial; host sums)

Batch-0's output projection is interleaved into batch-1's attention so
the PE chews o-proj matmuls while ACT runs the exp stream.
"""
import math
import numpy as np
import ml_dtypes

import concourse.bacc as bacc
import concourse.tile as tile
from concourse import mybir
from concourse.bass_utils import run_bass_kernel_spmd

F32 = mybir.dt.float32
F32R = mybir.dt.float32r
BF16 = mybir.dt.bfloat16

DT_PROJ = BF16
DT_QK = BF16
DT_ATT = BF16
NP_PROJ = ml_dtypes.bfloat16 if DT_PROJ == BF16 else np.float32
NP_ATT = ml_dtypes.bfloat16 if DT_ATT == BF16 else np.float32

P = 128
B, S, H = 2, 2048, 4096
T = B * S
DK = 128
NHL = 4
FL = NHL * DK
TB = 512
NTB = T // TB
NA = H // P
QBS = 512
NQB = S // QBS
NKT = S // P
SCALE = 1.0 / math.sqrt(DK)
NOB = H // 512
NTPB = S // P                # 16 output row tiles per batch

_NC_CACHE = {}


def build():
    nc = bacc.Bacc(None, target_bir_lowering=False)

    xt = nc.dram_tensor("xt", [H, T], DT_PROJ, kind="ExternalInput")
    wqt = nc.dram_tensor("wqt", [H, FL], DT_PROJ, kind="ExternalInput")
    wkt = nc.dram_tensor("wkt", [H, DK], DT_PROJ, kind="ExternalInput")
    wvt = nc.dram_tensor("wvt", [H, DK], DT_PROJ, kind="ExternalInput")
    wot = nc.dram_tensor("wot", [FL, H], DT_ATT, kind="ExternalInput")
    cost = nc.dram_tensor("cost", [P, S], F32, kind="ExternalInput")
    sints = nc.dram_tensor("sints", [P, S], F32, kind="ExternalInput")
    trimask = nc.dram_tensor("trimask", [P, P], BF16, kind="ExternalInput")
    identb = nc.dram_tensor("identb", [P, P], BF16, kind="ExternalInput")
    onesc = nc.dram_tensor("onesc", [P, 1], BF16, kind="ExternalInput")
    out = nc.dram_tensor("out", [T, H], F32, kind="ExternalOutput")

    EXP = mybir.ActivationFunctionType.Exp

    with nc.allow_low_precision(reason="attention compute dtypes are "
                                       "deliberately reduced"), \
         tile.TileContext(nc) as tc:
        with tc.tile_pool(name="const", bufs=1) as cp, \
             tc.tile_pool(name="dram", bufs=1, space="DRAM") as dp, \
             tc.tile_pool(name="attn", bufs=1) as ap, \
             tc.tile_pool(name="p2", bufs=1) as p2, \
             tc.tile_pool(name="p2q", bufs=2) as p2q, \
             tc.tile_pool(name="p2e", bufs=5) as p2e, \
             tc.tile_pool(name="p3w", bufs=2) as p3w, \
             tc.tile_pool(name="p3o", bufs=4) as p3o:
            cos_sb = cp.tile([P, S], F32)
            sin_sb = cp.tile([P, S], F32)
            tri_sb = cp.tile([P, P], BF16)
            id_sb = cp.tile([P, P], BF16)
            oc_sb = cp.tile([P, 1], BF16)
            nc.sync.dma_start(out=cos_sb, in_=cost[:, :])
            nc.sync.dma_start(out=sin_sb, in_=sints[:, :])
            nc.sync.dma_start(out=tri_sb, in_=trimask[:, :])
            nc.sync.dma_start(out=id_sb, in_=identb[:, :])
            nc.sync.dma_start(out=oc_sb, in_=onesc[:, :])

            attn_sb = [[ap.tile([P, S], DT_ATT, name=f"attn{b}_{h}")
                        for h in range(NHL)] for b in range(B)]
            q_scr = [dp.tile([FL, S], DT_QK, name=f"qscr{b}") for b in range(B)]
            k_scr = [dp.tile([DK, S], DT_QK, name=f"kscr{b}") for b in range(B)]
            v_scr = [dp.tile([DK, S], BF16, name=f"vscr{b}") for b in range(B)]

            # ---------------- Phase 1: QKV projection + RoPE ----------------
            with tc.tile_pool(name="wq", bufs=1) as wqp, \
                 tc.tile_pool(name="xp", bufs=10) as xp, \
                 tc.tile_pool(name="rp", bufs=1) as rp, \
                 tc.tile_pool(name="ps1", bufs=1, space="PSUM") as ps1:
                wq_sb = wqp.tile([P, NA * FL], DT_PROJ)
                wk_sb = wqp.tile([P, NA * DK], DT_PROJ)
                wv_sb = wqp.tile([P, NA * DK], DT_PROJ)
                for a in range(NA):
                    nc.sync.dma_start(out=wq_sb[:, a * FL:(a + 1) * FL],
                                      in_=wqt[a * P:(a + 1) * P, :])
                    nc.sync.dma_start(out=wk_sb[:, a * DK:(a + 1) * DK],
                                      in_=wkt[a * P:(a + 1) * P, :])
                    nc.sync.dma_start(out=wv_sb[:, a * DK:(a + 1) * DK],
                                      in_=wvt[a * P:(a + 1) * P, :])

                for tb in range(NTB):
                    bi = (tb * TB) // S
                    s0 = (tb * TB) % S
                    psq = [ps1.tile([P, TB], F32, name=f"psq{j}_{tb}",
                                    tag=f"psq{j}") for j in range(NHL)]
                    psk = ps1.tile([P, TB], F32, name=f"psk_{tb}", tag="psk")
                    psv = ps1.tile([P, TB], F32, name=f"psv_{tb}", tag="psv")
                    for a in range(NA):
                        xt_t = xp.tile([P, TB], DT_PROJ, name=f"x_{tb}_{a}",
                                       tag="xt")
                        nc.sync.dma_start(
                            out=xt_t,
                            in_=xt[a * P:(a + 1) * P, tb * TB:(tb + 1) * TB])
                        st, sp = (a == 0), (a == NA - 1)
                        nc.tensor.matmul(psk, wk_sb[:, a * DK:(a + 1) * DK],
                                         xt_t, start=st, stop=sp)
                        nc.tensor.matmul(psv, wv_sb[:, a * DK:(a + 1) * DK],
                                         xt_t, start=st, stop=sp)
                        for j in range(NHL):
                            nc.tensor.matmul(
                                psq[j],
                                wq_sb[:, a * FL + j * DK:a * FL + (j + 1) * DK],
                                xt_t, start=st, stop=sp)

                    # evict psum banks (one reader each, split ACT/DVE; K
                    # first since the next t-block's matmuls demand it first)
                    evs = []
                    plan = [(psk, k_scr, 0, nc.scalar),
                            (psq[0], q_scr, 0, nc.vector),
                            (psq[1], q_scr, P, nc.scalar),
                            (psq[2], q_scr, 2 * P, nc.vector),
                            (psq[3], q_scr, 3 * P, nc.scalar)]
                    for idx, (src, scr, r0, eng) in enumerate(plan):
                        qc = rp.tile([P, TB], F32, name=f"qc_{tb}_{idx}",
                                     tag="qc", bufs=7)
                        if eng is nc.scalar:
                            nc.scalar.copy(qc, src)
                        else:
                            nc.vector.tensor_copy(qc, src)
                        if idx == 0:
                            vb = rp.tile([P, TB], BF16, name=f"vb_{tb}",
                                         tag="vb", bufs=2)
                            nc.vector.tensor_copy(vb, psv)
                        evs.append((qc, scr, r0))
                    nc.scalar.dma_start(out=v_scr[bi][:, s0:s0 + TB], in_=vb)

                    # RoPE chains (SBUF only; eviction DMAs ride the ACT
                    # HWDGE ring so they never block the x-load stream)
                    for qc, scr, r0 in evs:
                        sw = rp.tile([P, TB], F32, name=f"sw_{tb}_{r0}",
                                     tag="sw", bufs=7)
                        nc.scalar.dma_start(out=sw[0:64, :], in_=qc[64:128, :])
                        nc.scalar.dma_start(out=sw[64:128, :], in_=qc[0:64, :])
                        nc.vector.tensor_mul(qc, qc, cos_sb[:, s0:s0 + TB])
                        nc.vector.tensor_mul(sw, sw, sin_sb[:, s0:s0 + TB])
                        qf = rp.tile([P, TB], DT_QK, name=f"qf_{tb}_{r0}",
                                     tag="qf", bufs=7)
                        nc.vector.tensor_add(qf, qc, sw)
                        nc.scalar.dma_start(
                            out=scr[bi][r0:r0 + P, s0:s0 + TB], in_=qf)

            # ------------- Phase 2 + interleaved output projection ----------
            with tc.tile_pool(name="ps2s", bufs=3, space="PSUM") as ps2s, \
                 tc.tile_pool(name="ps2u", bufs=2, space="PSUM") as ps2u:
                wo_tiles = {}
                ocnt = [0]

                def load_wo(ob):
                    wo_sb = p3w.tile([P, NHL, 512], DT_ATT,
                                     name=f"wo_{ob}_{ocnt[0]}", tag="wo",
                                     bufs=2)
                    for j in range(NHL):
                        nc.sync.dma_start(
                            out=wo_sb[:, j, :],
                            in_=wot[j * P:(j + 1) * P,
                                    ob * 512:(ob + 1) * 512])
                    wo_tiles[ob] = wo_sb

                def emit_otile(bt, ob, ti):
                    if ob not in wo_tiles:
                        load_wo(ob)
                    if ti == 4 and ob + 1 < NOB and (ob + 1) not in wo_tiles:
                        load_wo(ob + 1)
                    tt = bt * NTPB + ti
                    o_ps = ps2u.tile([P, 512], F32, name=f"o_{ocnt[0]}",
                                     tag="u", bufs=3)
                    ocnt[0] += 1
                    for j in range(NHL):
                        nc.tensor.matmul(
                            o_ps, attn_sb[bt][j][:, ti * P:(ti + 1) * P],
                            wo_tiles[ob][:, j, :],
                            start=(j == 0), stop=(j == NHL - 1))
                    o_sb = p3o.tile([P, 512], F32, name=f"os_{ocnt[0]}",
                                    tag="os")
                    nc.vector.tensor_copy(o_sb, o_ps)
                    nc.sync.dma_start(
                        out=out[tt * P:(tt + 1) * P, ob * 512:(ob + 1) * 512],
                        in_=o_sb)
                    if ti == NTPB - 1:
                        wo_tiles.pop(ob, None)

                def make_norm(b, h, qb, u_ps, d_ps):
                    def norm():
                        rf_sb = p2.tile([1, QBS], F32,
                                        name=f"rf_{b}_{h}_{qb}",
                                        tag="rf", bufs=2)
                        nc.vector.reciprocal_approx_fast(rf_sb, d_ps)
                        rb_sb = p2.tile([P, QBS], F32,
                                        name=f"rs_{b}_{h}_{qb}",
                                        tag="rs", bufs=2)
                        nc.gpsimd.partition_broadcast(rb_sb, rf_sb)
                        nc.vector.tensor_mul(
                            attn_sb[b][h][:, qb * QBS:(qb + 1) * QBS],
                            u_ps, rb_sb)
                    return norm

                # batch-0 o-proj tiles drip-fed into batch-1's attention
                inter = [(0, ob, ti) for ob in range(NOB)
                         for ti in range(NTPB)]
                inter_pos = 0

                pending = None
                mm_since = 0
                for b in range(B):
                    kb_sb = p2q.tile([P, S], DT_QK, name=f"kb_{b}", tag="kb")
                    nc.sync.dma_start(out=kb_sb, in_=k_scr[b][:, :])
                    vtk = p2q.tile([P, NKT, P], BF16, name=f"vt_{b}",
                                   tag="vtk")
                    nc.sync.dma_start_transpose(vtk, v_scr[b][:, :])
                    for h in range(NHL):
                        qh_sb = p2q.tile([P, S], DT_QK, name=f"q_{b}_{h}",
                                         tag="qh")
                        nc.sync.dma_start(out=qh_sb,
                                          in_=q_scr[b][h * P:(h + 1) * P, :])
                        for qb in range(NQB):
                            nkt = 4 * qb + 4
                            u_ps = ps2u.tile([P, QBS], F32,
                                             name=f"u_{b}_{h}_{qb}", tag="u",
                                             bufs=3)
                            d_ps = ps2u.tile([1, QBS], F32,
                                             name=f"d_{b}_{h}_{qb}", tag="d")

                            def emit_av(kt, e_sb, lo, u_ps=u_ps, d_ps=d_ps,
                                        nkt=nkt):
                                st, sp = (kt == 0), (kt == nkt - 1)
                                nc.tensor.matmul(u_ps[:, lo:], vtk[:, kt, :],
                                                 e_sb[:, lo:],
                                                 start=st, stop=sp,
                                                 skip_group_check=True)
                                nc.tensor.matmul(d_ps[:, lo:], oc_sb,
                                                 e_sb[:, lo:],
                                                 start=st, stop=sp,
                                                 skip_group_check=True)

                            av_fifo = []
                            for kt in range(nkt):
                                s_ps = ps2s.tile(
                                    [P, QBS], F32,
                                    name=f"s_{b}_{h}_{qb}_{kt}", tag="s")
                                m = kt - 4 * qb
                                lo = m * P if m > 0 else 0
                                nc.tensor.matmul(
                                    s_ps[:, lo:],
                                    kb_sb[:, kt * P:(kt + 1) * P],
                                    qh_sb[:, qb * QBS + lo:(qb + 1) * QBS],
                                    start=True, stop=True)
                                e_sb = p2e.tile(
                                    [P, QBS], BF16,
                                    name=f"e_{b}_{h}_{qb}_{kt}", tag="e")
                                nc.scalar.activation(e_sb[:, lo:],
                                                     s_ps[:, lo:], EXP,
                                                     scale=SCALE)
                                if m >= 0:
                                    nc.vector.tensor_mul(
                                        e_sb[:, m * P:(m + 1) * P],
                                        e_sb[:, m * P:(m + 1) * P],
                                        tri_sb)
                                if len(av_fifo) >= 3:
                                    emit_av(*av_fifo.pop(0))
                                av_fifo.append((kt, e_sb, lo))
                                mm_since += 3
                                if pending is not None and mm_since >= 12:
                                    pending()
                                    pending = None
                            for a0 in av_fifo:
                                emit_av(*a0)
                            pending = make_norm(b, h, qb, u_ps, d_ps)
                            mm_since = 0

                            # drip batch-0 o-proj into batch-1's attention
                            if b == 1:
                                for _ in range(8):
                                    if inter_pos < len(inter):
                                        emit_otile(*inter[inter_pos])
                                        inter_pos += 1
                pending()

                # leftover batch-0 tiles, then all of batch 1
                while inter_pos < len(inter):
                    emit_otile(*inter[inter_pos])
                    inter_pos += 1
                wo_tiles.clear()
                for ob in range(NOB):
                    for ti in range(NTPB):
                        emit_otile(1, ob, ti)

    nc.compile()
    return nc


def _prep_inputs(hidden_states, Wq, Wk, Wv, Wo, cos, sin):
    hs = np.asarray(hidden_states, dtype=np.float32)
    Wq = np.asarray(Wq, dtype=np.float32)
    Wk = np.asarray(Wk, dtype=np.float32)
    Wv = np.asarray(Wv, dtype=np.float32)
    Wo = np.asarray(Wo, dtype=np.float32)
    cos = np.asarray(cos, dtype=np.float32)
    sin = np.asarray(sin, dtype=np.float32)

    xt = np.ascontiguousarray(hs.reshape(T, H).T).astype(NP_PROJ)
    cosT = np.ascontiguousarray(cos.T)
    sinT = np.ascontiguousarray(sin.T)
    sints = np.ascontiguousarray(
        np.concatenate([-sinT[:64], sinT[64:]], axis=0))
    kq = np.arange(P)
    trim = (kq[None, :] >= kq[:, None]).astype(ml_dtypes.bfloat16)
    ident = np.eye(P, dtype=ml_dtypes.bfloat16)
    onesc = np.ones((P, 1), dtype=ml_dtypes.bfloat16)

    in_maps = []
    for c in range(8):
        in_maps.append({
            "xt": xt,
            "wqt": np.ascontiguousarray(
                Wq[c * FL:(c + 1) * FL, :].T).astype(NP_PROJ),
            "wkt": np.ascontiguousarray(
                Wk[c * DK:(c + 1) * DK, :].T).astype(NP_PROJ),
            "wvt": np.ascontiguousarray(
                Wv[c * DK:(c + 1) * DK, :].T).astype(NP_PROJ),
            "wot": np.ascontiguousarray(
                Wo[:, c * FL:(c + 1) * FL].T).astype(NP_ATT),
            "cost": cosT,
            "sints": sints,
            "trimask": trim,
            "identb": ident,
            "onesc": onesc,
        })
    return in_maps


def kernel(hidden_states, Wq, Wk, Wv, Wo, cos, sin, _run_kwargs=None):
    in_maps = _prep_inputs(hidden_states, Wq, Wk, Wv, Wo, cos, sin)
    if "nc" not in _NC_CACHE:
        _NC_CACHE["nc"] = build()
    nc = _NC_CACHE["nc"]
    kw = _run_kwargs or {}
    res = run_bass_kernel_spmd(nc, in_maps, core_ids=list(range(8)), **kw)
    acc = np.zeros((T, H), dtype=np.float64)
    for c in range(8):
        acc += np.asarray(res.results[c]["out"], dtype=np.float64)
    out = acc.astype(np.float32).reshape(B, S, H)
    if kw:
        _NC_CACHE["last_results"] = res
    return out



# revision 5
# speedup vs baseline: 1.0773x; 1.0773x over previous
"""Trainium2 Bass kernel for Llama GQA attention (B=2, S=2048, H=4096,
32 Q heads / 8 KV heads, head_dim 128, RoPE, causal).

Sharding: tensor-parallel by head across 8 cores. Core c owns Q heads
[4c..4c+3] and KV head c. Each core computes its Q/K/V projections,
RoPE, causal attention, and a partial output projection over its 512
attention features; the host sums the 8 partial outputs (bf16).

v2 layout decisions (vs the DRAM-scratch baseline):
  - q/k/v and attn live in SBUF end-to-end; no DRAM round trip, no
    phase-2 reload stall (which also caused a HAM re-throttle).
  - V is transposed to [tok, d] at eviction time with PE transposes.
  - Weight loads ride the ACT HWDGE ring as a few big 3D DMAs while
    the x-token stream owns the SP ring; first matmul starts ~5us in.
  - A short burst of dummy matmuls at t=0 warms the PE clock (HAM).
  - Phase 2 is qb-major: attention for all 4 heads of a 512-token
    q-block, then that block's 32 output-projection tiles, so o-proj
    DMA spreads across the whole phase instead of piling into a tail.
  - Output partials are written bf16 (halves output DMA bytes).
"""
import math
import numpy as np
import ml_dtypes

import concourse.bacc as bacc
import concourse.tile as tile
from concourse import mybir
from concourse.bass_utils import run_bass_kernel_spmd

F32 = mybir.dt.float32
BF16 = mybir.dt.bfloat16
NPBF = ml_dtypes.bfloat16

P = 128
B, S, H = 2, 2048, 4096
T = B * S
DK = 128
NHL = 4                      # Q heads per core
FL = NHL * DK                # 512 q features per core
TB = 512                     # token block in phase 1
NTB = T // TB
NA = H // P                  # 32 contraction slices
NAG = 4                      # a-slices per x DMA
QBS = 512                    # q block in phase 2
NQB = S // QBS
NKT = S // P
SCALE = 1.0 / math.sqrt(DK)
NOB = H // 512               # 8 output-column blocks

_NC_CACHE = {}


def build():
    nc = bacc.Bacc(None, target_bir_lowering=False)

    xt = nc.dram_tensor("xt", [H, T], BF16, kind="ExternalInput")
    wqt = nc.dram_tensor("wqt", [H, FL], BF16, kind="ExternalInput")
    wkt = nc.dram_tensor("wkt", [H, DK], BF16, kind="ExternalInput")
    wvt = nc.dram_tensor("wvt", [H, DK], BF16, kind="ExternalInput")
    wot = nc.dram_tensor("wot", [FL, H], BF16, kind="ExternalInput")
    cost = nc.dram_tensor("cost", [P, S], BF16, kind="ExternalInput")
    sints = nc.dram_tensor("sints", [P, S], BF16, kind="ExternalInput")
    trimask = nc.dram_tensor("trimask", [P, P], BF16, kind="ExternalInput")
    identb = nc.dram_tensor("identb", [P, P], BF16, kind="ExternalInput")
    onesc = nc.dram_tensor("onesc", [P, 1], BF16, kind="ExternalInput")
    out = nc.dram_tensor("out", [T, H], BF16, kind="ExternalOutput")

    EXP = mybir.ActivationFunctionType.Exp

    with nc.allow_low_precision(reason="attention compute dtypes are "
                                       "deliberately reduced"), \
         tile.TileContext(nc) as tc:
        with tc.tile_pool(name="const", bufs=1) as cp, \
             tc.tile_pool(name="resid", bufs=1) as rsp, \
             tc.tile_pool(name="wo", bufs=1) as wop:
            # ---- persistent SBUF tensors ----
            cos_sb = cp.tile([P, S], BF16)
            sin_sb = cp.tile([P, S], BF16)
            tri_sb = cp.tile([P, P], BF16)
            id_sb = cp.tile([P, P], BF16)
            oc_sb = cp.tile([P, 1], BF16)
            q_sb = [[rsp.tile([P, S], BF16, name=f"q{b}_{j}")
                     for j in range(NHL)] for b in range(B)]
            k_sb = [rsp.tile([P, S], BF16, name=f"k{b}") for b in range(B)]
            vtk_sb = [rsp.tile([P, NKT, P], BF16, name=f"vt{b}")
                      for b in range(B)]
            attn_sb = [[rsp.tile([P, S], BF16, name=f"attn{b}_{h}")
                        for h in range(NHL)] for b in range(B)]
            wo_sb = wop.tile([P, NHL, H], BF16)

            # ---- PE clock warm-up: dummy matmuls with no DMA deps ----
            with tc.tile_pool(name="warm", bufs=1) as wp, \
                 tc.tile_pool(name="warmp", bufs=1, space="PSUM") as wpp:
                dum = wp.tile([P, TB], BF16)
                nc.vector.memset(dum, 0.0)
                wps = wpp.tile([P, TB], F32)
                for i in range(10):
                    nc.tensor.matmul(wps, dum[:, :P], dum,
                                     start=True, stop=True)

            # ---- bulk loads: weights on ACT ring, wo on gpsimd ring ----
            wk_sb = cp.tile([P, NA, DK], BF16)
            wv_sb = cp.tile([P, NA, DK], BF16)
            wq_sb = cp.tile([P, NA, FL], BF16)
            nc.scalar.dma_start(
                out=wk_sb, in_=wkt[:, :].rearrange("(a p) f -> p a f", p=P))
            nc.scalar.dma_start(
                out=wv_sb, in_=wvt[:, :].rearrange("(a p) f -> p a f", p=P))
            wq_view = wqt[:, :].rearrange("(a p) f -> p a f", p=P)
            for g in range(4):
                nc.scalar.dma_start(out=wq_sb[:, g * 8:(g + 1) * 8, :],
                                    in_=wq_view[:, g * 8:(g + 1) * 8, :])
            nc.scalar.dma_start(out=tri_sb, in_=trimask[:, :])
            nc.scalar.dma_start(out=id_sb, in_=identb[:, :])
            nc.scalar.dma_start(out=oc_sb, in_=onesc[:, :])
            nc.scalar.dma_start(out=cos_sb, in_=cost[:, :])
            nc.scalar.dma_start(out=sin_sb, in_=sints[:, :])
            nc.gpsimd.dma_start(
                out=wo_sb, in_=wot[:, :].rearrange("(j p) o -> p j o", p=P))

            # ---------------- Phase 1: QKV projection + RoPE --------------
            xt_view = xt[:, :].rearrange("(a p) t -> p a t", p=P)
            with tc.tile_pool(name="xp", bufs=3) as xp, \
                 tc.tile_pool(name="rp", bufs=1) as rp, \
                 tc.tile_pool(name="ps1", bufs=1, space="PSUM") as ps1:

                def rope(src, dst, s0, uid):
                    # dst = src*cos + swap_halves(src)*sints  (all bf16)
                    sw = rp.tile([P, TB], BF16, name=f"sw_{uid}",
                                 tag="sw", bufs=6)
                    nc.scalar.dma_start(out=sw[0:64, :], in_=src[64:128, :])
                    nc.scalar.dma_start(out=sw[64:128, :], in_=src[0:64, :])
                    nc.vector.tensor_mul(src, src, cos_sb[:, s0:s0 + TB])
                    nc.vector.tensor_mul(sw, sw, sin_sb[:, s0:s0 + TB])
                    nc.vector.tensor_add(dst, src, sw)

                for tb in range(NTB):
                    bi = (tb * TB) // S
                    s0 = (tb * TB) % S
                    psq = [ps1.tile([P, TB], F32, name=f"psq{j}_{tb}",
                                    tag=f"psq{j}") for j in range(NHL)]
                    psk = ps1.tile([P, TB], F32, name=f"psk_{tb}", tag="psk")
                    psv = ps1.tile([P, TB], F32, name=f"psv_{tb}", tag="psv")
                    for ag in range(NA // NAG):
                        x_t = xp.tile([P, NAG, TB], BF16,
                                      name=f"x_{tb}_{ag}", tag="xt")
                        nc.sync.dma_start(
                            out=x_t,
                            in_=xt_view[:, ag * NAG:(ag + 1) * NAG,
                                        tb * TB:(tb + 1) * TB])
                        for ai in range(NAG):
                            a = ag * NAG + ai
                            st, sp = (a == 0), (a == NA - 1)
                            nc.tensor.matmul(psk, wk_sb[:, a, :],
                                             x_t[:, ai, :], start=st, stop=sp)
                            nc.tensor.matmul(psv, wv_sb[:, a, :],
                                             x_t[:, ai, :], start=st, stop=sp)
                            for j in range(NHL):
                                nc.tensor.matmul(
                                    psq[j],
                                    wq_sb[:, a, j * DK:(j + 1) * DK],
                                    x_t[:, ai, :], start=st, stop=sp)

                    # evictions: K first (next tb's first matmul is psk)
                    kc = rp.tile([P, TB], BF16, name=f"kc_{tb}", tag="kc",
                                 bufs=2)
                    nc.scalar.copy(kc, psk)
                    rope(kc, k_sb[bi][:, s0:s0 + TB], s0, f"k{tb}")
                    vb = rp.tile([P, TB], BF16, name=f"vb_{tb}", tag="vb",
                                 bufs=2)
                    nc.vector.tensor_copy(vb, psv)
                    vt_ps = ps1.tile([P, TB], BF16, name=f"vt_{tb}", tag="vt")
                    for m in range(4):
                        nc.tensor.transpose(vt_ps[:, m * P:(m + 1) * P],
                                            vb[:, m * P:(m + 1) * P], id_sb)
                    kt0 = (s0 // P)
                    nc.scalar.copy(vtk_sb[bi][:, kt0:kt0 + 4, :], vt_ps)
                    for j in range(NHL):
                        qc = rp.tile([P, TB], BF16, name=f"qc_{tb}_{j}",
                                     tag="qc", bufs=6)
                        if j % 2 == 0:
                            nc.vector.tensor_copy(qc, psq[j])
                        else:
                            nc.scalar.copy(qc, psq[j])
                        rope(qc, q_sb[bi][j][:, s0:s0 + TB], s0, f"q{tb}_{j}")

            # ------------- Phase 2: attention + output projection ---------
            with tc.tile_pool(name="p2", bufs=1) as p2, \
                 tc.tile_pool(name="p2e", bufs=5) as p2e, \
                 tc.tile_pool(name="p3o", bufs=4) as p3o, \
                 tc.tile_pool(name="ps2s", bufs=3, space="PSUM") as ps2s, \
                 tc.tile_pool(name="ps2u", bufs=3, space="PSUM") as ps2u:
                ocnt = [0]

                def emit_otile(b, ti, ob):
                    o_ps = ps2u.tile([P, 512], F32, name=f"o_{ocnt[0]}",
                                     tag="u", bufs=3)
                    for j in range(NHL):
                        nc.tensor.matmul(
                            o_ps, attn_sb[b][j][:, ti * P:(ti + 1) * P],
                            wo_sb[:, j, ob * 512:(ob + 1) * 512],
                            start=(j == 0), stop=(j == NHL - 1))
                    o_sb = p3o.tile([P, 512], BF16, name=f"os_{ocnt[0]}",
                                    tag="os")
                    if ocnt[0] % 2 == 0:
                        nc.vector.tensor_copy(o_sb, o_ps)
                    else:
                        nc.scalar.copy(o_sb, o_ps)
                    eng = nc.sync if ocnt[0] % 2 == 0 else nc.scalar
                    r0 = b * S + ti * P
                    eng.dma_start(out=out[r0:r0 + P, ob * 512:(ob + 1) * 512],
                                  in_=o_sb)
                    ocnt[0] += 1

                for b in range(B):
                    for qb in range(NQB):
                        nkt = 4 * qb + 4
                        for h in range(NHL):
                            u_ps = ps2u.tile([P, QBS], F32,
                                             name=f"u_{b}_{h}_{qb}", tag="u",
                                             bufs=3)
                            d_ps = ps2u.tile([1, QBS], F32,
                                             name=f"d_{b}_{h}_{qb}", tag="d",
                                             bufs=2)

                            def emit_av(kt, e_sb, lo, u_ps=u_ps, d_ps=d_ps,
                                        nkt=nkt, b=b):
                                st, sp = (kt == 0), (kt == nkt - 1)
                                nc.tensor.matmul(u_ps[:, lo:],
                                                 vtk_sb[b][:, kt, :],
                                                 e_sb[:, lo:],
                                                 start=st, stop=sp,
                                                 skip_group_check=True)
                                nc.tensor.matmul(d_ps[:, lo:], oc_sb,
                                                 e_sb[:, lo:],
                                                 start=st, stop=sp,
                                                 skip_group_check=True)

                            av_fifo = []
                            for kt in range(nkt):
                                s_ps = ps2s.tile(
                                    [P, QBS], F32,
                                    name=f"s_{b}_{h}_{qb}_{kt}", tag="s")
                                m = kt - 4 * qb
                                lo = m * P if m > 0 else 0
                                nc.tensor.matmul(
                                    s_ps[:, lo:],
                                    k_sb[b][:, kt * P:(kt + 1) * P],
                                    q_sb[b][h][:, qb * QBS + lo:
                                               (qb + 1) * QBS],
                                    start=True, stop=True)
                                e_sb = p2e.tile(
                                    [P, QBS], BF16,
                                    name=f"e_{b}_{h}_{qb}_{kt}", tag="e")
                                nc.scalar.activation(e_sb[:, lo:],
                                                     s_ps[:, lo:], EXP,
                                                     scale=SCALE)
                                if m >= 0:
                                    nc.vector.tensor_mul(
                                        e_sb[:, m * P:(m + 1) * P],
                                        e_sb[:, m * P:(m + 1) * P],
                                        tri_sb)
                                if len(av_fifo) >= 3:
                                    emit_av(*av_fifo.pop(0))
                                av_fifo.append((kt, e_sb, lo))
                            for a0 in av_fifo:
                                emit_av(*a0)

                            # normalize into attn_sb (off the PE path)
                            rf_sb = p2.tile([1, QBS], F32,
                                            name=f"rf_{b}_{h}_{qb}",
                                            tag="rf", bufs=2)
                            nc.vector.reciprocal_approx_fast(rf_sb, d_ps)
                            rb_sb = p2.tile([P, QBS], F32,
                                            name=f"rs_{b}_{h}_{qb}",
                                            tag="rs", bufs=2)
                            nc.gpsimd.partition_broadcast(rb_sb, rf_sb)
                            nc.vector.tensor_mul(
                                attn_sb[b][h][:, qb * QBS:(qb + 1) * QBS],
                                u_ps, rb_sb)

                        # this q-block's output tiles (all heads now done)
                        for i in range(4):
                            for ob in range(NOB):
                                emit_otile(b, qb * 4 + i, ob)

    nc.compile()
    return nc


def _prep_inputs(hidden_states, Wq, Wk, Wv, Wo, cos, sin):
    hs = np.asarray(hidden_states, dtype=np.float32)
    Wq = np.asarray(Wq, dtype=np.float32)
    Wk = np.asarray(Wk, dtype=np.float32)
    Wv = np.asarray(Wv, dtype=np.float32)
    Wo = np.asarray(Wo, dtype=np.float32)
    cos = np.asarray(cos, dtype=np.float32)
    sin = np.asarray(sin, dtype=np.float32)

    xtm = np.ascontiguousarray(hs.reshape(T, H).T).astype(NPBF)
    cosT = np.ascontiguousarray(cos.T).astype(NPBF)
    sinT = np.ascontiguousarray(sin.T)
    sints = np.ascontiguousarray(
        np.concatenate([-sinT[:64], sinT[64:]], axis=0)).astype(NPBF)
    kq = np.arange(P)
    trim = (kq[None, :] >= kq[:, None]).astype(NPBF)
    ident = np.eye(P, dtype=NPBF)
    onesc = np.ones((P, 1), dtype=NPBF)

    in_maps = []
    for c in range(8):
        in_maps.append({
            "xt": xtm,
            "wqt": np.ascontiguousarray(
                Wq[c * FL:(c + 1) * FL, :].T).astype(NPBF),
            "wkt": np.ascontiguousarray(
                Wk[c * DK:(c + 1) * DK, :].T).astype(NPBF),
            "wvt": np.ascontiguousarray(
                Wv[c * DK:(c + 1) * DK, :].T).astype(NPBF),
            "wot": np.ascontiguousarray(
                Wo[:, c * FL:(c + 1) * FL].T).astype(NPBF),
            "cost": cosT,
            "sints": sints,
            "trimask": trim,
            "identb": ident,
            "onesc": onesc,
        })
    return in_maps


def kernel(hidden_states, Wq, Wk, Wv, Wo, cos, sin, _run_kwargs=None):
    in_maps = _prep_inputs(hidden_states, Wq, Wk, Wv, Wo, cos, sin)
    if "nc" not in _NC_CACHE:
        _NC_CACHE["nc"] = build()
    nc = _NC_CACHE["nc"]
    kw = _run_kwargs or {}
    res = run_bass_kernel_spmd(nc, in_maps, core_ids=list(range(8)), **kw)
    acc = np.zeros((T, H), dtype=np.float64)
    for c in range(8):
        acc += np.asarray(res.results[c]["out"], dtype=np.float64)
    out = acc.astype(np.float32).reshape(B, S, H)
    if kw:
        _NC_CACHE["last_results"] = res
    return out


# revision 12
# speedup vs baseline: 1.1284x; 1.0474x over previous
"""Trainium2 Bass kernel for Llama GQA attention (B=2, S=2048, H=4096,
32 Q heads / 8 KV heads, head_dim 128, RoPE, causal).

Sharding: tensor-parallel by head across 8 cores. Core c owns Q heads
[4c..4c+3] and KV head c. Each core computes its Q/K/V projections,
RoPE, causal attention, and a partial output projection over its 512
attention features; the host sums the 8 partial outputs (bf16).

v2 layout decisions (vs the DRAM-scratch baseline):
  - q/k/v and attn live in SBUF end-to-end; no DRAM round trip, no
    phase-2 reload stall (which also caused a HAM re-throttle).
  - V is transposed to [tok, d] at eviction time with PE transposes.
  - Weight loads ride the ACT HWDGE ring as a few big 3D DMAs while
    the x-token stream owns the SP ring; first matmul starts ~5us in.
  - A short burst of dummy matmuls at t=0 warms the PE clock (HAM).
  - Phase 2 is qb-major: attention for all 4 heads of a 512-token
    q-block, then that block's 32 output-projection tiles, so o-proj
    DMA spreads across the whole phase instead of piling into a tail.
  - Output partials are written bf16 (halves output DMA bytes).
"""
import math
import numpy as np
import ml_dtypes

import concourse.bacc as bacc
import concourse.tile as tile
from concourse import mybir
from concourse.bass_utils import run_bass_kernel_spmd

F32 = mybir.dt.float32
BF16 = mybir.dt.bfloat16
NPBF = ml_dtypes.bfloat16

P = 128
B, S, H = 2, 2048, 4096
T = B * S
DK = 128
NHL = 4                      # Q heads per core
FL = NHL * DK                # 512 q features per core
TB = 512                     # token block in phase 1
NTB = T // TB
NA = H // P                  # 32 contraction slices
NAG = 4                      # a-slices per x DMA
QBS = 512                    # q block in phase 2
NQB = S // QBS
NKT = S // P
SCALE = 1.0 / math.sqrt(DK)
NOB = H // 512               # 8 output-column blocks

_NC_CACHE = {}


def build():
    nc = bacc.Bacc(None, target_bir_lowering=False)

    xt = nc.dram_tensor("xt", [H, T], BF16, kind="ExternalInput")
    wqt = nc.dram_tensor("wqt", [H, FL], BF16, kind="ExternalInput")
    wkt = nc.dram_tensor("wkt", [H, DK], BF16, kind="ExternalInput")
    wvt = nc.dram_tensor("wvt", [H, DK], BF16, kind="ExternalInput")
    wot = nc.dram_tensor("wot", [FL, H], BF16, kind="ExternalInput")
    cost = nc.dram_tensor("cost", [P, S], BF16, kind="ExternalInput")
    sints = nc.dram_tensor("sints", [P, S], BF16, kind="ExternalInput")
    trimask = nc.dram_tensor("trimask", [P, P], BF16, kind="ExternalInput")
    identb = nc.dram_tensor("identb", [P, P], BF16, kind="ExternalInput")
    onesc = nc.dram_tensor("onesc", [P, 1], BF16, kind="ExternalInput")
    out = nc.dram_tensor("out", [T, H], BF16, kind="ExternalOutput")

    EXP = mybir.ActivationFunctionType.Exp

    with nc.allow_low_precision(reason="attention compute dtypes are "
                                       "deliberately reduced"), \
         tile.TileContext(nc) as tc:
        with tc.tile_pool(name="const", bufs=1) as cp, \
             tc.tile_pool(name="resid", bufs=1) as rsp, \
             tc.tile_pool(name="wo", bufs=1) as wop:
            # ---- persistent SBUF tensors ----
            cos_sb = cp.tile([P, S], BF16)
            sin_sb = cp.tile([P, S], BF16)
            tri_sb = cp.tile([P, P], BF16)
            id_sb = cp.tile([P, P], BF16)
            oc_sb = cp.tile([P, 1], BF16)
            q_sb = [[rsp.tile([P, S], BF16, name=f"q{b}_{j}")
                     for j in range(NHL)] for b in range(B)]
            k_sb = [rsp.tile([P, S], BF16, name=f"k{b}") for b in range(B)]
            vtk_sb = [rsp.tile([P, NKT, P], BF16, name=f"vt{b}")
                      for b in range(B)]
            attn_sb = [[rsp.tile([P, S], BF16, name=f"attn{b}_{h}")
                        for h in range(NHL)] for b in range(B)]
            wo_sb = wop.tile([P, NHL, H], BF16)

            # ---- PE clock warm-up: dummy matmuls with no DMA deps ----
            with tc.tile_pool(name="warm", bufs=1) as wp, \
                 tc.tile_pool(name="warmp", bufs=1, space="PSUM") as wpp:
                dum = wp.tile([P, TB], BF16)
                nc.vector.memset(dum, 0.0)
                wps = wpp.tile([P, TB], F32)
                for i in range(10):
                    nc.tensor.matmul(wps, dum[:, :P], dum,
                                     start=True, stop=True)

            # ---- bulk loads on the ACT ring, in need-order: the first
            # token block needs wk/wv/wq[a] progressively, then the
            # eviction/RoPE chain needs id + cos/sin. wo rides the gpsimd
            # ring mid-phase-1 so it doesn't contend at startup.
            wk_sb = cp.tile([P, NA, DK], BF16)
            wv_sb = cp.tile([P, NA, DK], BF16)
            wq_sb = cp.tile([P, NA, FL], BF16)
            wk_view = wkt[:, :].rearrange("(a p) f -> p a f", p=P)
            wv_view = wvt[:, :].rearrange("(a p) f -> p a f", p=P)
            wq_view = wqt[:, :].rearrange("(a p) f -> p a f", p=P)
            nc.scalar.dma_start(out=wk_sb[:, :16, :], in_=wk_view[:, :16, :])
            nc.scalar.dma_start(out=wv_sb[:, :16, :], in_=wv_view[:, :16, :])
            nc.scalar.dma_start(out=wq_sb[:, :8, :], in_=wq_view[:, :8, :])
            nc.scalar.dma_start(out=wk_sb[:, 16:, :], in_=wk_view[:, 16:, :])
            nc.scalar.dma_start(out=wv_sb[:, 16:, :], in_=wv_view[:, 16:, :])
            nc.scalar.dma_start(out=id_sb, in_=identb[:, :])
            nc.scalar.dma_start(out=oc_sb, in_=onesc[:, :])
            nc.scalar.dma_start(out=cos_sb, in_=cost[:, :])
            nc.scalar.dma_start(out=sin_sb, in_=sints[:, :])
            for g in range(1, 4):
                nc.scalar.dma_start(out=wq_sb[:, g * 8:(g + 1) * 8, :],
                                    in_=wq_view[:, g * 8:(g + 1) * 8, :])
            nc.scalar.dma_start(out=tri_sb, in_=trimask[:, :])

            # ---------------- Phase 1: QKV projection + RoPE --------------
            xt_view = xt[:, :].rearrange("(a p) t -> p a t", p=P)
            with tc.tile_pool(name="xp", bufs=3) as xp, \
                 tc.tile_pool(name="rp", bufs=1) as rp, \
                 tc.tile_pool(name="ps1", bufs=1, space="PSUM") as ps1:

                def rope(src, dst, s0, uid):
                    # dst = src*cos + swap_halves(src)*sints  (all bf16)
                    sw = rp.tile([P, TB], BF16, name=f"sw_{uid}",
                                 tag="sw", bufs=6)
                    nc.gpsimd.dma_start(out=sw[0:64, :], in_=src[64:128, :])
                    nc.gpsimd.dma_start(out=sw[64:128, :], in_=src[0:64, :])
                    nc.vector.tensor_mul(src, src, cos_sb[:, s0:s0 + TB])
                    nc.vector.tensor_mul(sw, sw, sin_sb[:, s0:s0 + TB])
                    nc.vector.tensor_add(dst, src, sw)

                for tb in range(NTB):
                    if tb == 1:
                        # 4MB wo load rides the otherwise-idle gpsimd ring
                        nc.gpsimd.dma_start(
                            out=wo_sb,
                            in_=wot[:, :].rearrange("(j p) o -> p j o", p=P))
                    bi = (tb * TB) // S
                    s0 = (tb * TB) % S
                    psq = [ps1.tile([P, TB], F32, name=f"psq{j}_{tb}",
                                    tag=f"psq{j}") for j in range(NHL)]
                    psk = ps1.tile([P, TB], F32, name=f"psk_{tb}", tag="psk")
                    psv = ps1.tile([P, TB], F32, name=f"psv_{tb}", tag="psv")
                    for ag in range(NA // NAG):
                        x_t = xp.tile([P, NAG, TB], BF16,
                                      name=f"x_{tb}_{ag}", tag="xt")
                        nc.sync.dma_start(
                            out=x_t,
                            in_=xt_view[:, ag * NAG:(ag + 1) * NAG,
                                        tb * TB:(tb + 1) * TB])
                        for ai in range(NAG):
                            a = ag * NAG + ai
                            st, sp = (a == 0), (a == NA - 1)
                            nc.tensor.matmul(psk, wk_sb[:, a, :],
                                             x_t[:, ai, :], start=st, stop=sp)
                            nc.tensor.matmul(psv, wv_sb[:, a, :],
                                             x_t[:, ai, :], start=st, stop=sp)
                            for j in range(NHL):
                                nc.tensor.matmul(
                                    psq[j],
                                    wq_sb[:, a, j * DK:(j + 1) * DK],
                                    x_t[:, ai, :], start=st, stop=sp)

                    # evictions: K first (next tb's first matmul is psk)
                    kc = rp.tile([P, TB], BF16, name=f"kc_{tb}", tag="kc",
                                 bufs=2)
                    nc.scalar.copy(kc, psk)
                    rope(kc, k_sb[bi][:, s0:s0 + TB], s0, f"k{tb}")
                    vb = rp.tile([P, TB], BF16, name=f"vb_{tb}", tag="vb",
                                 bufs=2)
                    nc.vector.tensor_copy(vb, psv)
                    vt_ps = ps1.tile([P, TB], BF16, name=f"vt_{tb}", tag="vt")
                    for m in range(4):
                        nc.tensor.transpose(vt_ps[:, m * P:(m + 1) * P],
                                            vb[:, m * P:(m + 1) * P], id_sb)
                    kt0 = (s0 // P)
                    nc.scalar.copy(vtk_sb[bi][:, kt0:kt0 + 4, :], vt_ps)
                    for j in range(NHL):
                        qc = rp.tile([P, TB], BF16, name=f"qc_{tb}_{j}",
                                     tag="qc", bufs=6)
                        if j % 2 == 0:
                            nc.vector.tensor_copy(qc, psq[j])
                        else:
                            nc.scalar.copy(qc, psq[j])
                        rope(qc, q_sb[bi][j][:, s0:s0 + TB], s0, f"q{tb}_{j}")

            # ------------- Phase 2: attention + output projection ---------
            # qb runs 3..0 so the phase starts with the deepest kt
            # pipeline (absorbs the tb7 eviction/RoPE epilogue latency).
            # Full (non-diagonal) exp tiles are pre-reduced 4:1 on the DVE
            # so the softmax-denominator matmuls stream 4x fewer columns.
            with tc.tile_pool(name="p2", bufs=1) as p2, \
                 tc.tile_pool(name="p2e", bufs=5) as p2e, \
                 tc.tile_pool(name="p3o", bufs=3) as p3o, \
                 tc.tile_pool(name="ps2s", bufs=3, space="PSUM") as ps2s, \
                 tc.tile_pool(name="ps2u", bufs=3, space="PSUM") as ps2u:
                ocnt = [0]

                def emit_otile(b, ti, ob2):
                    # one [128, 1024] tile covering output blocks 2*ob2,
                    # 2*ob2+1; a single paired DMA on the SP ring
                    o_sb = p3o.tile([P, 1024], BF16, name=f"os_{ocnt[0]}",
                                    tag="os")
                    for half in range(2):
                        ob = 2 * ob2 + half
                        o_ps = ps2u.tile([P, 512], F32,
                                         name=f"o_{ocnt[0]}_{half}",
                                         tag="u", bufs=3)
                        for j in range(NHL):
                            nc.tensor.matmul(
                                o_ps, attn_sb[b][j][:, ti * P:(ti + 1) * P],
                                wo_sb[:, j, ob * 512:(ob + 1) * 512],
                                start=(j == 0), stop=(j == NHL - 1))
                        dst = o_sb[:, half * 512:(half + 1) * 512]
                        if (ocnt[0] + half) % 2 == 0:
                            nc.vector.tensor_copy(dst, o_ps)
                        else:
                            nc.scalar.copy(dst, o_ps)
                    r0 = b * S + ti * P
                    nc.sync.dma_start(
                        out=out[r0:r0 + P, ob2 * 1024:(ob2 + 1) * 1024],
                        in_=o_sb)
                    ocnt[0] += 1

                for b in range(B):
                    for qb in range(NQB - 1, -1, -1):
                        nkt = 4 * qb + 4
                        for h in range(NHL):
                            u_ps = ps2u.tile([P, QBS], F32,
                                             name=f"u_{b}_{h}_{qb}", tag="u",
                                             bufs=3)
                            d_ps = ps2u.tile([1, QBS], F32,
                                             name=f"d_{b}_{h}_{qb}", tag="d",
                                             bufs=2)
                            dflag = [True]

                            def emit_d(src, lo, sp, d_ps=d_ps):
                                nc.tensor.matmul(d_ps[:, lo:], oc_sb,
                                                 src[:, lo:],
                                                 start=dflag[0], stop=sp,
                                                 skip_group_check=True)
                                dflag[0] = False

                            def emit_av(kt, e_sb, lo, u_ps=u_ps, nkt=nkt,
                                        b=b):
                                st, sp = (kt == 0), (kt == nkt - 1)
                                nc.tensor.matmul(u_ps[:, lo:],
                                                 vtk_sb[b][:, kt, :],
                                                 e_sb[:, lo:],
                                                 start=st, stop=sp,
                                                 skip_group_check=True)

                            av_fifo = []
                            diag_e = []   # diagonal e tiles (post-mask)
                            fulls = []    # full e tiles awaiting pair add
                            pairs = []    # pair sums awaiting quad add
                            quads = []    # (ready_kt, quad tile) for d-mm
                            for kt in range(nkt):
                                s_ps = ps2s.tile(
                                    [P, QBS], F32,
                                    name=f"s_{b}_{h}_{qb}_{kt}", tag="s")
                                m = kt - 4 * qb
                                lo = m * P if m > 0 else 0
                                nc.tensor.matmul(
                                    s_ps[:, lo:],
                                    k_sb[b][:, kt * P:(kt + 1) * P],
                                    q_sb[b][h][:, qb * QBS + lo:
                                               (qb + 1) * QBS],
                                    start=True, stop=True)
                                e_sb = p2e.tile(
                                    [P, QBS], BF16,
                                    name=f"e_{b}_{h}_{qb}_{kt}", tag="e")
                                nc.scalar.activation(e_sb[:, lo:],
                                                     s_ps[:, lo:], EXP,
                                                     scale=SCALE)
                                if m >= 0:
                                    nc.vector.tensor_mul(
                                        e_sb[:, m * P:(m + 1) * P],
                                        e_sb[:, m * P:(m + 1) * P],
                                        tri_sb)
                                    diag_e.append(e_sb)
                                else:
                                    # 4:1 DVE pre-reduction of full tiles
                                    # for the denominator matmul
                                    fulls.append(e_sb)
                                    if len(fulls) == 2:
                                        es = p2.tile(
                                            [P, QBS], BF16,
                                            name=f"ep_{b}_{h}_{qb}_{kt}",
                                            tag="ep", bufs=3)
                                        nc.vector.tensor_add(
                                            es, fulls[0], fulls[1])
                                        fulls = []
                                        pairs.append(es)
                                        if len(pairs) == 2:
                                            eq = p2.tile(
                                                [P, QBS], BF16,
                                                name=f"eq_{b}_{h}_{qb}_{kt}",
                                                tag="eq", bufs=2)
                                            nc.vector.tensor_add(
                                                eq, pairs[0], pairs[1])
                                            pairs = []
                                            quads.append((kt + 3, eq))
                                if len(av_fifo) >= 3:
                                    emit_av(*av_fifo.pop(0))
                                av_fifo.append((kt, e_sb, lo))
                                if quads and kt >= quads[0][0]:
                                    emit_d(quads.pop(0)[1], 0, False)
                            for a0 in av_fifo:
                                emit_av(*a0)
                            for _, eq in quads:
                                emit_d(eq, 0, False)
                            for dk in range(4):
                                emit_d(diag_e[dk], dk * P, dk == 3)

                            # normalize into attn_sb (off the PE path)
                            rf_sb = p2.tile([1, QBS], F32,
                                            name=f"rf_{b}_{h}_{qb}",
                                            tag="rf", bufs=2)
                            nc.vector.reciprocal_approx_fast(rf_sb, d_ps)
                            rb_sb = p2.tile([P, QBS], F32,
                                            name=f"rs_{b}_{h}_{qb}",
                                            tag="rs", bufs=2)
                            nc.gpsimd.partition_broadcast(rb_sb, rf_sb)
                            nc.vector.tensor_mul(
                                attn_sb[b][h][:, qb * QBS:(qb + 1) * QBS],
                                u_ps, rb_sb)

                        # this q-block's output tiles (all heads now done)
                        for i in range(4):
                            for ob2 in range(NOB // 2):
                                emit_otile(b, qb * 4 + i, ob2)

    nc.compile()
    return nc


def _prep_inputs(hidden_states, Wq, Wk, Wv, Wo, cos, sin):
    hs = np.asarray(hidden_states, dtype=np.float32)
    Wq = np.asarray(Wq, dtype=np.float32)
    Wk = np.asarray(Wk, dtype=np.float32)
    Wv = np.asarray(Wv, dtype=np.float32)
    Wo = np.asarray(Wo, dtype=np.float32)
    cos = np.asarray(cos, dtype=np.float32)
    sin = np.asarray(sin, dtype=np.float32)

    xtm = np.ascontiguousarray(hs.reshape(T, H).T).astype(NPBF)
    cosT = np.ascontiguousarray(cos.T).astype(NPBF)
    sinT = np.ascontiguousarray(sin.T)
    sints = np.ascontiguousarray(
        np.concatenate([-sinT[:64], sinT[64:]], axis=0)).astype(NPBF)
    kq = np.arange(P)
    trim = (kq[None, :] >= kq[:, None]).astype(NPBF)
    ident = np.eye(P, dtype=NPBF)
    onesc = np.ones((P, 1), dtype=NPBF)

    in_maps = []
    for c in range(8):
        in_maps.append({
            "xt": xtm,
            "wqt": np.ascontiguousarray(
                Wq[c * FL:(c + 1) * FL, :].T).astype(NPBF),
            "wkt": np.ascontiguousarray(
                Wk[c * DK:(c + 1) * DK, :].T).astype(NPBF),
            "wvt": np.ascontiguousarray(
                Wv[c * DK:(c + 1) * DK, :].T).astype(NPBF),
            "wot": np.ascontiguousarray(
                Wo[:, c * FL:(c + 1) * FL].T).astype(NPBF),
            "cost": cosT,
            "sints": sints,
            "trimask": trim,
            "identb": ident,
            "onesc": onesc,
        })
    return in_maps


def kernel(hidden_states, Wq, Wk, Wv, Wo, cos, sin, _run_kwargs=None):
    in_maps = _prep_inputs(hidden_states, Wq, Wk, Wv, Wo, cos, sin)
    if "nc" not in _NC_CACHE:
        _NC_CACHE["nc"] = build()
    nc = _NC_CACHE["nc"]
    kw = _run_kwargs or {}
    res = run_bass_kernel_spmd(nc, in_maps, core_ids=list(range(8)), **kw)
    acc = np.zeros((T, H), dtype=np.float64)
    for c in range(8):
        acc += np.asarray(res.results[c]["out"], dtype=np.float64)
    out = acc.astype(np.float32).reshape(B, S, H)
    if kw:
        _NC_CACHE["last_results"] = res
    return out


# revision 18
# speedup vs baseline: 1.1584x; 1.0266x over previous
"""Trainium2 Bass kernel for Llama GQA attention (B=2, S=2048, H=4096,
32 Q heads / 8 KV heads, head_dim 128, RoPE, causal).

Sharding: tensor-parallel by head across 8 cores. Core c owns Q heads
[4c..4c+3] and KV head c. Each core computes its Q/K/V projections,
RoPE, causal attention, and a partial output projection over its 512
attention features; the host sums the 8 partial outputs (bf16).

v2 layout decisions (vs the DRAM-scratch baseline):
  - q/k/v and attn live in SBUF end-to-end; no DRAM round trip, no
    phase-2 reload stall (which also caused a HAM re-throttle).
  - V is transposed to [tok, d] at eviction time with PE transposes.
  - Weight loads ride the ACT HWDGE ring as a few big 3D DMAs while
    the x-token stream owns the SP ring; first matmul starts ~5us in.
  - A short burst of dummy matmuls at t=0 warms the PE clock (HAM).
  - Phase 2 is qb-major: attention for all 4 heads of a 512-token
    q-block, then that block's 32 output-projection tiles, so o-proj
    DMA spreads across the whole phase instead of piling into a tail.
  - Output partials are written bf16 (halves output DMA bytes).
"""
import math
import numpy as np
import ml_dtypes

import concourse.bacc as bacc
import concourse.tile as tile
from concourse import mybir
from concourse.bass_utils import run_bass_kernel_spmd

F32 = mybir.dt.float32
BF16 = mybir.dt.bfloat16
NPBF = ml_dtypes.bfloat16

P = 128
B, S, H = 2, 2048, 4096
T = B * S
DK = 128
NHL = 4                      # Q heads per core
FL = NHL * DK                # 512 q features per core
TB = 512                     # token block in phase 1
NTB = T // TB
NA = H // P                  # 32 contraction slices
NAG = 4                      # a-slices per x DMA
QBS = 512                    # q block in phase 2
NQB = S // QBS
NKT = S // P
SCALE = 1.0 / math.sqrt(DK)
NOB = H // 512               # 8 output-column blocks

_NC_CACHE = {}


def build():
    nc = bacc.Bacc(None, target_bir_lowering=False)

    # weights arrive pre-transposed to partition-major SBUF layout so each
    # load is one DMA with large contiguous per-partition descriptors
    xt = nc.dram_tensor("xt", [H, T], BF16, kind="ExternalInput")
    wqt = nc.dram_tensor("wqt", [P, NA * FL], BF16, kind="ExternalInput")
    wkt = nc.dram_tensor("wkt", [P, NA * DK], BF16, kind="ExternalInput")
    wvt = nc.dram_tensor("wvt", [P, NA * DK], BF16, kind="ExternalInput")
    wot = nc.dram_tensor("wot", [P, NHL * H], BF16, kind="ExternalInput")
    cost = nc.dram_tensor("cost", [P, S], BF16, kind="ExternalInput")
    sints = nc.dram_tensor("sints", [P, S], BF16, kind="ExternalInput")
    trimask = nc.dram_tensor("trimask", [P, P], BF16, kind="ExternalInput")
    identb = nc.dram_tensor("identb", [P, P], BF16, kind="ExternalInput")
    onesc = nc.dram_tensor("onesc", [P, 1], BF16, kind="ExternalInput")
    out = nc.dram_tensor("out", [T, H], BF16, kind="ExternalOutput")

    EXP = mybir.ActivationFunctionType.Exp

    with nc.allow_low_precision(reason="attention compute dtypes are "
                                       "deliberately reduced"), \
         tile.TileContext(nc) as tc:
        with tc.tile_pool(name="const", bufs=1) as cp, \
             tc.tile_pool(name="resid", bufs=1) as rsp, \
             tc.tile_pool(name="wo", bufs=1) as wop:
            # ---- persistent SBUF tensors ----
            cos_sb = cp.tile([P, S], BF16)
            sin_sb = cp.tile([P, S], BF16)
            tri_sb = cp.tile([P, P], BF16)
            id_sb = cp.tile([P, P], BF16)
            oc_sb = cp.tile([P, 1], BF16)
            q_sb = [[rsp.tile([P, S], BF16, name=f"q{b}_{j}")
                     for j in range(NHL)] for b in range(B)]
            k_sb = [rsp.tile([P, S], BF16, name=f"k{b}") for b in range(B)]
            vtk_sb = [rsp.tile([P, NKT, P], BF16, name=f"vt{b}")
                      for b in range(B)]
            attn_sb = [[rsp.tile([P, S], BF16, name=f"attn{b}_{h}")
                        for h in range(NHL)] for b in range(B)]
            wo_sb = wop.tile([P, NHL, H], BF16)

            # ---- bulk loads on the ACT ring, in need-order: the first
            # token block needs wk/wv/wq[a] progressively, then the
            # eviction/RoPE chain needs id + cos/sin. wo rides the gpsimd
            # ring mid-phase-1 so it doesn't contend at startup.
            wk_sb = cp.tile([P, NA, DK], BF16)
            wv_sb = cp.tile([P, NA, DK], BF16)
            wq_sb = cp.tile([P, NA, FL], BF16)
            wk_view = wkt[:, :].rearrange("p (a f) -> p a f", a=NA)
            wv_view = wvt[:, :].rearrange("p (a f) -> p a f", a=NA)
            wq_view = wqt[:, :].rearrange("p (a f) -> p a f", a=NA)
            nc.scalar.dma_start(out=wk_sb, in_=wk_view)
            nc.scalar.dma_start(out=wv_sb, in_=wv_view)
            nc.scalar.dma_start(out=wq_sb[:, :8, :], in_=wq_view[:, :8, :])
            nc.scalar.dma_start(out=id_sb, in_=identb[:, :])
            nc.scalar.dma_start(out=oc_sb, in_=onesc[:, :])
            nc.scalar.dma_start(out=cos_sb, in_=cost[:, :])
            nc.scalar.dma_start(out=sin_sb, in_=sints[:, :])
            for g in range(1, 4):
                nc.scalar.dma_start(out=wq_sb[:, g * 8:(g + 1) * 8, :],
                                    in_=wq_view[:, g * 8:(g + 1) * 8, :])
            nc.scalar.dma_start(out=tri_sb, in_=trimask[:, :])

            # ---------------- Phase 1: QKV projection + RoPE --------------
            xt_view = xt[:, :].rearrange("(a p) t -> p a t", p=P)
            with tc.tile_pool(name="xp", bufs=3) as xp, \
                 tc.tile_pool(name="rp", bufs=1) as rp, \
                 tc.tile_pool(name="ps1", bufs=1, space="PSUM") as ps1:

                def rope(src, dst, s0, uid):
                    # dst = src*cos + swap_halves(src)*sints  (all bf16)
                    sw = rp.tile([P, TB], BF16, name=f"sw_{uid}",
                                 tag="sw", bufs=6)
                    nc.gpsimd.dma_start(out=sw[0:64, :], in_=src[64:128, :])
                    nc.gpsimd.dma_start(out=sw[64:128, :], in_=src[0:64, :])
                    nc.vector.tensor_mul(src, src, cos_sb[:, s0:s0 + TB])
                    nc.vector.tensor_mul(sw, sw, sin_sb[:, s0:s0 + TB])
                    nc.vector.tensor_add(dst, src, sw)

                for tb in range(NTB):
                    if tb == 1:
                        # 4MB wo load rides the otherwise-idle gpsimd ring
                        nc.gpsimd.dma_start(
                            out=wo_sb,
                            in_=wot[:, :].rearrange("p (j o) -> p j o",
                                                    j=NHL))
                    bi = (tb * TB) // S
                    s0 = (tb * TB) % S
                    psq = [ps1.tile([P, TB], F32, name=f"psq{j}_{tb}",
                                    tag=f"psq{j}") for j in range(NHL)]
                    psk = ps1.tile([P, TB], F32, name=f"psk_{tb}", tag="psk")
                    psv = ps1.tile([P, TB], F32, name=f"psv_{tb}", tag="psv")
                    for ag in range(NA // NAG):
                        x_t = xp.tile([P, NAG, TB], BF16,
                                      name=f"x_{tb}_{ag}", tag="xt")
                        nc.sync.dma_start(
                            out=x_t,
                            in_=xt_view[:, ag * NAG:(ag + 1) * NAG,
                                        tb * TB:(tb + 1) * TB])
                        for ai in range(NAG):
                            a = ag * NAG + ai
                            st, sp = (a == 0), (a == NA - 1)
                            nc.tensor.matmul(psk, wk_sb[:, a, :],
                                             x_t[:, ai, :], start=st, stop=sp)
                            nc.tensor.matmul(psv, wv_sb[:, a, :],
                                             x_t[:, ai, :], start=st, stop=sp)
                            for j in range(NHL):
                                nc.tensor.matmul(
                                    psq[j],
                                    wq_sb[:, a, j * DK:(j + 1) * DK],
                                    x_t[:, ai, :], start=st, stop=sp)

                    # evictions: K first (next tb's first matmul is psk)
                    kc = rp.tile([P, TB], BF16, name=f"kc_{tb}", tag="kc",
                                 bufs=2)
                    nc.scalar.copy(kc, psk)
                    rope(kc, k_sb[bi][:, s0:s0 + TB], s0, f"k{tb}")
                    vb = rp.tile([P, TB], BF16, name=f"vb_{tb}", tag="vb",
                                 bufs=2)
                    nc.vector.tensor_copy(vb, psv)
                    vt_ps = ps1.tile([P, TB], BF16, name=f"vt_{tb}", tag="vt")
                    for m in range(4):
                        nc.tensor.transpose(vt_ps[:, m * P:(m + 1) * P],
                                            vb[:, m * P:(m + 1) * P], id_sb)
                    kt0 = (s0 // P)
                    nc.scalar.copy(vtk_sb[bi][:, kt0:kt0 + 4, :], vt_ps)
                    for j in range(NHL):
                        qc = rp.tile([P, TB], BF16, name=f"qc_{tb}_{j}",
                                     tag="qc", bufs=6)
                        if j % 2 == 0:
                            nc.vector.tensor_copy(qc, psq[j])
                        else:
                            nc.scalar.copy(qc, psq[j])
                        rope(qc, q_sb[bi][j][:, s0:s0 + TB], s0, f"q{tb}_{j}")

            # ------------- Phase 2: attention + output projection ---------
            # qb runs 3..0 so the phase starts with the deepest kt
            # pipeline (absorbs the tb7 eviction/RoPE epilogue latency).
            # Full (non-diagonal) exp tiles are pre-reduced 4:1 on the DVE
            # so the softmax-denominator matmuls stream 4x fewer columns.
            with tc.tile_pool(name="p2", bufs=1) as p2, \
                 tc.tile_pool(name="p2e", bufs=5) as p2e, \
                 tc.tile_pool(name="p3o", bufs=3) as p3o, \
                 tc.tile_pool(name="ps2s", bufs=3, space="PSUM") as ps2s, \
                 tc.tile_pool(name="ps2u", bufs=3, space="PSUM") as ps2u:
                ocnt = [0]

                def emit_otile(b, ti, ob2):
                    # one [128, 1024] tile covering output blocks 2*ob2,
                    # 2*ob2+1; a single paired DMA on the SP ring
                    o_sb = p3o.tile([P, 1024], BF16, name=f"os_{ocnt[0]}",
                                    tag="os")
                    for half in range(2):
                        ob = 2 * ob2 + half
                        o_ps = ps2u.tile([P, 512], F32,
                                         name=f"o_{ocnt[0]}_{half}",
                                         tag="u", bufs=3)
                        for j in range(NHL):
                            nc.tensor.matmul(
                                o_ps, attn_sb[b][j][:, ti * P:(ti + 1) * P],
                                wo_sb[:, j, ob * 512:(ob + 1) * 512],
                                start=(j == 0), stop=(j == NHL - 1))
                        dst = o_sb[:, half * 512:(half + 1) * 512]
                        if (ocnt[0] + half) % 2 == 0:
                            nc.vector.tensor_copy(dst, o_ps)
                        else:
                            nc.scalar.copy(dst, o_ps)
                    r0 = b * S + ti * P
                    nc.sync.dma_start(
                        out=out[r0:r0 + P, ob2 * 1024:(ob2 + 1) * 1024],
                        in_=o_sb)
                    ocnt[0] += 1

                # o-proj tiles are produced per finished q-block and
                # consumed one-per-kt inside later attention blocks, so
                # PE fills the slack while ACT streams the exps
                pend = []
                for b in range(B):
                    for qb in range(NQB - 1, -1, -1):
                        nkt = 4 * qb + 4
                        for h in range(NHL):
                            u_ps = ps2u.tile([P, QBS], F32,
                                             name=f"u_{b}_{h}_{qb}", tag="u",
                                             bufs=3)
                            d_ps = ps2u.tile([1, QBS], F32,
                                             name=f"d_{b}_{h}_{qb}", tag="d",
                                             bufs=2)
                            dflag = [True]

                            def emit_d(src, lo, sp, d_ps=d_ps):
                                nc.tensor.matmul(d_ps[:, lo:], oc_sb,
                                                 src[:, lo:],
                                                 start=dflag[0], stop=sp,
                                                 skip_group_check=True)
                                dflag[0] = False

                            def emit_av(kt, e_sb, lo, u_ps=u_ps, nkt=nkt,
                                        b=b):
                                st, sp = (kt == 0), (kt == nkt - 1)
                                nc.tensor.matmul(u_ps[:, lo:],
                                                 vtk_sb[b][:, kt, :],
                                                 e_sb[:, lo:],
                                                 start=st, stop=sp,
                                                 skip_group_check=True)

                            av_fifo = []
                            diag_e = []   # diagonal e tiles (post-mask)
                            fulls = []    # full e tiles awaiting pair add
                            pairs = []    # pair sums awaiting quad add
                            quads = []    # (ready_kt, quad tile) for d-mm
                            for kt in range(nkt):
                                s_ps = ps2s.tile(
                                    [P, QBS], F32,
                                    name=f"s_{b}_{h}_{qb}_{kt}", tag="s")
                                m = kt - 4 * qb
                                lo = m * P if m > 0 else 0
                                nc.tensor.matmul(
                                    s_ps[:, lo:],
                                    k_sb[b][:, kt * P:(kt + 1) * P],
                                    q_sb[b][h][:, qb * QBS + lo:
                                               (qb + 1) * QBS],
                                    start=True, stop=True)
                                e_sb = p2e.tile(
                                    [P, QBS], BF16,
                                    name=f"e_{b}_{h}_{qb}_{kt}", tag="e")
                                nc.scalar.activation(e_sb[:, lo:],
                                                     s_ps[:, lo:], EXP,
                                                     scale=SCALE)
                                if m >= 0:
                                    nc.vector.tensor_mul(
                                        e_sb[:, m * P:(m + 1) * P],
                                        e_sb[:, m * P:(m + 1) * P],
                                        tri_sb)
                                    diag_e.append(e_sb)
                                else:
                                    # 4:1 DVE pre-reduction of full tiles
                                    # for the denominator matmul
                                    fulls.append(e_sb)
                                    if len(fulls) == 2:
                                        es = p2.tile(
                                            [P, QBS], BF16,
                                            name=f"ep_{b}_{h}_{qb}_{kt}",
                                            tag="ep", bufs=3)
                                        nc.vector.tensor_add(
                                            es, fulls[0], fulls[1])
                                        fulls = []
                                        pairs.append(es)
                                        if len(pairs) == 2:
                                            eq = p2.tile(
                                                [P, QBS], BF16,
                                                name=f"eq_{b}_{h}_{qb}_{kt}",
                                                tag="eq", bufs=2)
                                            nc.vector.tensor_add(
                                                eq, pairs[0], pairs[1])
                                            pairs = []
                                            quads.append((kt + 3, eq))
                                if len(av_fifo) >= 3:
                                    emit_av(*av_fifo.pop(0))
                                av_fifo.append((kt, e_sb, lo))
                                if quads and kt >= quads[0][0]:
                                    emit_d(quads.pop(0)[1], 0, False)
                                if pend:
                                    emit_otile(*pend.pop(0))
                            # close the denominator group and start the
                            # normalization chain before draining the AV
                            # fifo, so 1/D is ready when u_ps closes
                            for _, eq in quads:
                                emit_d(eq, 0, False)
                            for dk in range(4):
                                emit_d(diag_e[dk], dk * P, dk == 3)
                            rf_sb = p2.tile([1, QBS], F32,
                                            name=f"rf_{b}_{h}_{qb}",
                                            tag="rf", bufs=2)
                            nc.vector.reciprocal_approx_fast(rf_sb, d_ps)
                            rb_sb = p2.tile([P, QBS], F32,
                                            name=f"rs_{b}_{h}_{qb}",
                                            tag="rs", bufs=2)
                            nc.gpsimd.partition_broadcast(rb_sb, rf_sb)
                            for a0 in av_fifo:
                                emit_av(*a0)
                            nc.vector.tensor_mul(
                                attn_sb[b][h][:, qb * QBS:(qb + 1) * QBS],
                                u_ps, rb_sb)

                        # queue this q-block's output tiles (heads done)
                        for i in range(4):
                            for ob2 in range(NOB // 2):
                                pend.append((b, qb * 4 + i, ob2))
                # drain whatever o-proj work is still queued
                for args in pend:
                    emit_otile(*args)

    nc.compile()
    return nc


def _prep_inputs(hidden_states, Wq, Wk, Wv, Wo, cos, sin):
    hs = np.asarray(hidden_states, dtype=np.float32)
    Wq = np.asarray(Wq, dtype=np.float32)
    Wk = np.asarray(Wk, dtype=np.float32)
    Wv = np.asarray(Wv, dtype=np.float32)
    Wo = np.asarray(Wo, dtype=np.float32)
    cos = np.asarray(cos, dtype=np.float32)
    sin = np.asarray(sin, dtype=np.float32)

    xtm = np.ascontiguousarray(hs.reshape(T, H).T).astype(NPBF)
    cosT = np.ascontiguousarray(cos.T).astype(NPBF)
    sinT = np.ascontiguousarray(sin.T)
    sints = np.ascontiguousarray(
        np.concatenate([-sinT[:64], sinT[64:]], axis=0)).astype(NPBF)
    kq = np.arange(P)
    trim = (kq[None, :] >= kq[:, None]).astype(NPBF)
    ident = np.eye(P, dtype=NPBF)
    onesc = np.ones((P, 1), dtype=NPBF)

    def pmajor(w):
        # [rows, cols] -> [128, (rows/128)*cols] partition-major layout
        r, ccols = w.shape
        return np.ascontiguousarray(
            w.reshape(r // P, P, ccols).transpose(1, 0, 2).reshape(P, -1)
        ).astype(NPBF)

    in_maps = []
    for c in range(8):
        in_maps.append({
            "xt": xtm,
            "wqt": pmajor(np.ascontiguousarray(Wq[c * FL:(c + 1) * FL, :].T)),
            "wkt": pmajor(np.ascontiguousarray(Wk[c * DK:(c + 1) * DK, :].T)),
            "wvt": pmajor(np.ascontiguousarray(Wv[c * DK:(c + 1) * DK, :].T)),
            "wot": pmajor(np.ascontiguousarray(Wo[:, c * FL:(c + 1) * FL].T)),
            "cost": cosT,
            "sints": sints,
            "trimask": trim,
            "identb": ident,
            "onesc": onesc,
        })
    return in_maps


def kernel(hidden_states, Wq, Wk, Wv, Wo, cos, sin, _run_kwargs=None):
    in_maps = _prep_inputs(hidden_states, Wq, Wk, Wv, Wo, cos, sin)
    if "nc" not in _NC_CACHE:
        _NC_CACHE["nc"] = build()
    nc = _NC_CACHE["nc"]
    kw = _run_kwargs or {}
    res = run_bass_kernel_spmd(nc, in_maps, core_ids=list(range(8)), **kw)
    acc = np.zeros((T, H), dtype=np.float64)
    for c in range(8):
        acc += np.asarray(res.results[c]["out"], dtype=np.float64)
    out = acc.astype(np.float32).reshape(B, S, H)
    if kw:
        _NC_CACHE["last_results"] = res
    return out


# revision 25
# speedup vs baseline: 1.1621x; 1.0032x over previous
"""Trainium2 Bass kernel for Llama GQA attention (B=2, S=2048, H=4096,
32 Q heads / 8 KV heads, head_dim 128, RoPE, causal).

Sharding: tensor-parallel by head across 8 cores. Core c owns Q heads
[4c..4c+3] and KV head c. Each core computes its Q/K/V projections,
RoPE, causal attention, and a partial output projection over its 512
attention features; the host sums the 8 partial outputs (bf16).

v2 layout decisions (vs the DRAM-scratch baseline):
  - q/k/v and attn live in SBUF end-to-end; no DRAM round trip, no
    phase-2 reload stall (which also caused a HAM re-throttle).
  - V is transposed to [tok, d] at eviction time with PE transposes.
  - Weight loads ride the ACT HWDGE ring as a few big 3D DMAs while
    the x-token stream owns the SP ring; first matmul starts ~5us in.
  - A short burst of dummy matmuls at t=0 warms the PE clock (HAM).
  - Phase 2 is qb-major: attention for all 4 heads of a 512-token
    q-block, then that block's 32 output-projection tiles, so o-proj
    DMA spreads across the whole phase instead of piling into a tail.
  - Output partials are written bf16 (halves output DMA bytes).
"""
import math
import numpy as np
import ml_dtypes

import concourse.bacc as bacc
import concourse.tile as tile
from concourse import mybir
from concourse.bass_utils import run_bass_kernel_spmd

F32 = mybir.dt.float32
BF16 = mybir.dt.bfloat16
NPBF = ml_dtypes.bfloat16

P = 128
B, S, H = 2, 2048, 4096
T = B * S
DK = 128
NHL = 4                      # Q heads per core
FL = NHL * DK                # 512 q features per core
TB = 512                     # token block in phase 1
NTB = T // TB
NA = H // P                  # 32 contraction slices
NAG = 4                      # a-slices per x DMA
QBS = 512                    # q block in phase 2
NQB = S // QBS
NKT = S // P
SCALE = 1.0 / math.sqrt(DK)
NOB = H // 512               # 8 output-column blocks

_NC_CACHE = {}


def build():
    nc = bacc.Bacc(None, target_bir_lowering=False)

    # weights and x arrive pre-transposed to partition-major SBUF layout so
    # each load is one DMA with large contiguous per-partition descriptors
    # (small-descriptor streams starve under packet-level SDMA round-robin)
    xt = nc.dram_tensor("xt", [NTB * P, NA * TB], BF16, kind="ExternalInput")
    wqt = nc.dram_tensor("wqt", [P, NA * FL], BF16, kind="ExternalInput")
    wkt = nc.dram_tensor("wkt", [P, NA * DK], BF16, kind="ExternalInput")
    wvt = nc.dram_tensor("wvt", [P, NA * DK], BF16, kind="ExternalInput")
    wot = nc.dram_tensor("wot", [P, NHL * H], BF16, kind="ExternalInput")
    cost = nc.dram_tensor("cost", [P, S], BF16, kind="ExternalInput")
    sints = nc.dram_tensor("sints", [P, S], BF16, kind="ExternalInput")
    trimask = nc.dram_tensor("trimask", [P, P], BF16, kind="ExternalInput")
    identb = nc.dram_tensor("identb", [P, P], BF16, kind="ExternalInput")
    onesc = nc.dram_tensor("onesc", [P, 1], BF16, kind="ExternalInput")
    out = nc.dram_tensor("out", [T, H], BF16, kind="ExternalOutput")

    EXP = mybir.ActivationFunctionType.Exp

    with nc.allow_low_precision(reason="attention compute dtypes are "
                                       "deliberately reduced"), \
         tile.TileContext(nc) as tc:
        with tc.tile_pool(name="const", bufs=1) as cp, \
             tc.tile_pool(name="resid", bufs=1) as rsp, \
             tc.tile_pool(name="wo", bufs=1) as wop:
            # ---- persistent SBUF tensors ----
            cos_sb = cp.tile([P, S], BF16)
            sin_sb = cp.tile([P, S], BF16)
            tri_sb = cp.tile([P, P], BF16)
            id_sb = cp.tile([P, P], BF16)
            oc_sb = cp.tile([P, 1], BF16)
            q_sb = [[rsp.tile([P, S], BF16, name=f"q{b}_{j}")
                     for j in range(NHL)] for b in range(B)]
            k_sb = [rsp.tile([P, S], BF16, name=f"k{b}") for b in range(B)]
            vtk_sb = [rsp.tile([P, NKT, P], BF16, name=f"vt{b}")
                      for b in range(B)]
            attn_sb = [[rsp.tile([P, S], BF16, name=f"attn{b}_{h}")
                        for h in range(NHL)] for b in range(B)]
            wo_sb = wop.tile([P, NHL, H], BF16)

            # ---- bulk loads on the ACT ring, in need-order: the first
            # token block needs wk/wv/wq[a] progressively, then the
            # eviction/RoPE chain needs id + cos/sin. wo rides the gpsimd
            # ring mid-phase-1 so it doesn't contend at startup.
            wk_sb = cp.tile([P, NA, DK], BF16)
            wv_sb = cp.tile([P, NA, DK], BF16)
            wq_sb = cp.tile([P, NA, FL], BF16)
            wk_view = wkt[:, :].rearrange("p (a f) -> p a f", a=NA)
            wv_view = wvt[:, :].rearrange("p (a f) -> p a f", a=NA)
            wq_view = wqt[:, :].rearrange("p (a f) -> p a f", a=NA)
            nc.scalar.dma_start(out=wk_sb, in_=wk_view)
            nc.scalar.dma_start(out=wv_sb, in_=wv_view)
            nc.scalar.dma_start(out=wq_sb[:, :8, :], in_=wq_view[:, :8, :])
            nc.scalar.dma_start(out=id_sb, in_=identb[:, :])
            nc.scalar.dma_start(out=oc_sb, in_=onesc[:, :])
            nc.scalar.dma_start(out=cos_sb, in_=cost[:, :])
            nc.scalar.dma_start(out=sin_sb, in_=sints[:, :])
            for g in range(1, 4):
                nc.scalar.dma_start(out=wq_sb[:, g * 8:(g + 1) * 8, :],
                                    in_=wq_view[:, g * 8:(g + 1) * 8, :])
            nc.scalar.dma_start(out=tri_sb, in_=trimask[:, :])

            # ---------------- Phase 1: QKV projection + RoPE --------------
            with tc.tile_pool(name="xp", bufs=2) as xp, \
                 tc.tile_pool(name="rp", bufs=1) as rp, \
                 tc.tile_pool(name="ps1", bufs=1, space="PSUM") as ps1:

                def rope(src, dst, s0, uid):
                    # dst = src*cos + swap_halves(src)*sints  (all bf16)
                    sw = rp.tile([P, TB], BF16, name=f"sw_{uid}",
                                 tag="sw", bufs=6)
                    nc.gpsimd.dma_start(out=sw[0:64, :], in_=src[64:128, :])
                    nc.gpsimd.dma_start(out=sw[64:128, :], in_=src[0:64, :])
                    nc.vector.tensor_mul(src, src, cos_sb[:, s0:s0 + TB])
                    nc.vector.tensor_mul(sw, sw, sin_sb[:, s0:s0 + TB])
                    nc.vector.tensor_add(dst, src, sw)

                for tb in range(NTB):
                    if tb == 1:
                        # 4MB wo load on the gpsimd ring; the tiny copy
                        # into its corner pins it behind tb0 (the Tile
                        # scheduler would otherwise hoist it to t=0 where
                        # it starves the startup-critical loads)
                        nc.gpsimd.tensor_copy(wo_sb[0:1, 0, 0:2],
                                              k_sb[0][0:1, 0:2])
                        nc.gpsimd.dma_start(
                            out=wo_sb,
                            in_=wot[:, :].rearrange("p (j o) -> p j o",
                                                    j=NHL))
                    bi = (tb * TB) // S
                    s0 = (tb * TB) % S
                    psq = [ps1.tile([P, TB], F32, name=f"psq{j}_{tb}",
                                    tag=f"psq{j}") for j in range(NHL)]
                    psk = ps1.tile([P, TB], F32, name=f"psk_{tb}", tag="psk")
                    psv = ps1.tile([P, TB], F32, name=f"psv_{tb}", tag="psv")
                    for ag in range(NA // 8):
                        x_t = xp.tile([P, 8, TB], BF16,
                                      name=f"x_{tb}_{ag}", tag="xt")
                        nc.sync.dma_start(
                            out=x_t,
                            in_=xt[tb * P:(tb + 1) * P,
                                   ag * 8 * TB:(ag + 1) * 8 * TB].rearrange(
                                       "p (a t) -> p a t", a=8))
                        for ai in range(8):
                            a = ag * 8 + ai
                            st, sp = (a == 0), (a == NA - 1)
                            nc.tensor.matmul(psk, wk_sb[:, a, :],
                                             x_t[:, ai, :], start=st, stop=sp)
                            nc.tensor.matmul(psv, wv_sb[:, a, :],
                                             x_t[:, ai, :], start=st, stop=sp)
                            for j in range(NHL):
                                nc.tensor.matmul(
                                    psq[j],
                                    wq_sb[:, a, j * DK:(j + 1) * DK],
                                    x_t[:, ai, :], start=st, stop=sp)

                    # evictions: K first (next tb's first matmul is psk)
                    kc = rp.tile([P, TB], BF16, name=f"kc_{tb}", tag="kc",
                                 bufs=2)
                    nc.scalar.copy(kc, psk)
                    rope(kc, k_sb[bi][:, s0:s0 + TB], s0, f"k{tb}")
                    vb = rp.tile([P, TB], BF16, name=f"vb_{tb}", tag="vb",
                                 bufs=2)
                    nc.vector.tensor_copy(vb, psv)
                    vt_ps = ps1.tile([P, TB], BF16, name=f"vt_{tb}", tag="vt")
                    for m in range(4):
                        nc.tensor.transpose(vt_ps[:, m * P:(m + 1) * P],
                                            vb[:, m * P:(m + 1) * P], id_sb)
                    kt0 = (s0 // P)
                    nc.scalar.copy(vtk_sb[bi][:, kt0:kt0 + 4, :], vt_ps)
                    for j in range(NHL):
                        qc = rp.tile([P, TB], BF16, name=f"qc_{tb}_{j}",
                                     tag="qc", bufs=6)
                        if j % 2 == 0:
                            nc.vector.tensor_copy(qc, psq[j])
                        else:
                            nc.scalar.copy(qc, psq[j])
                        rope(qc, q_sb[bi][j][:, s0:s0 + TB], s0, f"q{tb}_{j}")

            # ------------- Phase 2: attention + output projection ---------
            # qb runs 3..0 so the phase starts with the deepest kt
            # pipeline (absorbs the tb7 eviction/RoPE epilogue latency).
            # Full (non-diagonal) exp tiles are pre-reduced 4:1 on the DVE
            # so the softmax-denominator matmuls stream 4x fewer columns.
            with tc.tile_pool(name="p2", bufs=1) as p2, \
                 tc.tile_pool(name="p2e", bufs=5) as p2e, \
                 tc.tile_pool(name="p3o", bufs=4) as p3o, \
                 tc.tile_pool(name="ps2s", bufs=3, space="PSUM") as ps2s, \
                 tc.tile_pool(name="ps2u", bufs=3, space="PSUM") as ps2u:
                ocnt = [0]

                def emit_otile(b, ti, ob2):
                    # one [128, 1024] tile covering output blocks 2*ob2,
                    # 2*ob2+1; a single paired DMA on the SP ring
                    o_sb = p3o.tile([P, 1024], BF16, name=f"os_{ocnt[0]}",
                                    tag="os")
                    for half in range(2):
                        ob = 2 * ob2 + half
                        o_ps = ps2u.tile([P, 512], F32,
                                         name=f"o_{ocnt[0]}_{half}",
                                         tag="u", bufs=3)
                        for j in range(NHL):
                            nc.tensor.matmul(
                                o_ps, attn_sb[b][j][:, ti * P:(ti + 1) * P],
                                wo_sb[:, j, ob * 512:(ob + 1) * 512],
                                start=(j == 0), stop=(j == NHL - 1))
                        dst = o_sb[:, half * 512:(half + 1) * 512]
                        if (ocnt[0] + half) % 2 == 0:
                            nc.vector.tensor_copy(dst, o_ps)
                        else:
                            nc.scalar.copy(dst, o_ps)
                    r0 = b * S + ti * P
                    eng = nc.sync if ocnt[0] % 2 == 0 else nc.scalar
                    eng.dma_start(
                        out=out[r0:r0 + P, ob2 * 1024:(ob2 + 1) * 1024],
                        in_=o_sb)
                    ocnt[0] += 1

                # o-proj tiles are produced per finished q-block and
                # consumed one-per-kt inside later attention blocks, so
                # PE fills the slack while ACT streams the exps
                pend = []
                for b in range(B):
                    for qb in range(NQB - 1, -1, -1):
                        nkt = 4 * qb + 4
                        for h in range(NHL):
                            u_ps = ps2u.tile([P, QBS], F32,
                                             name=f"u_{b}_{h}_{qb}", tag="u",
                                             bufs=3)
                            d_ps = ps2u.tile([1, QBS], F32,
                                             name=f"d_{b}_{h}_{qb}", tag="d",
                                             bufs=2)
                            dflag = [True]

                            def emit_d(src, lo, sp, d_ps=d_ps):
                                nc.tensor.matmul(d_ps[:, lo:], oc_sb,
                                                 src[:, lo:],
                                                 start=dflag[0], stop=sp,
                                                 skip_group_check=True)
                                dflag[0] = False

                            def emit_av(kt, e_sb, lo, u_ps=u_ps, nkt=nkt,
                                        b=b):
                                st, sp = (kt == 0), (kt == nkt - 1)
                                nc.tensor.matmul(u_ps[:, lo:],
                                                 vtk_sb[b][:, kt, :],
                                                 e_sb[:, lo:],
                                                 start=st, stop=sp,
                                                 skip_group_check=True)

                            av_fifo = []
                            diag_e = []   # diagonal e tiles (post-mask)
                            fulls = []    # full e tiles awaiting pair add
                            pairs = []    # pair sums awaiting quad add
                            quads = []    # (ready_kt, quad tile) for d-mm
                            for kt in range(nkt):
                                s_ps = ps2s.tile(
                                    [P, QBS], F32,
                                    name=f"s_{b}_{h}_{qb}_{kt}", tag="s")
                                m = kt - 4 * qb
                                lo = m * P if m > 0 else 0
                                nc.tensor.matmul(
                                    s_ps[:, lo:],
                                    k_sb[b][:, kt * P:(kt + 1) * P],
                                    q_sb[b][h][:, qb * QBS + lo:
                                               (qb + 1) * QBS],
                                    start=True, stop=True)
                                e_sb = p2e.tile(
                                    [P, QBS], BF16,
                                    name=f"e_{b}_{h}_{qb}_{kt}", tag="e")
                                nc.scalar.activation(e_sb[:, lo:],
                                                     s_ps[:, lo:], EXP,
                                                     scale=SCALE)
                                if m >= 0:
                                    nc.vector.tensor_mul(
                                        e_sb[:, m * P:(m + 1) * P],
                                        e_sb[:, m * P:(m + 1) * P],
                                        tri_sb)
                                    diag_e.append(e_sb)
                                else:
                                    # 4:1 DVE pre-reduction of full tiles
                                    # for the denominator matmul
                                    fulls.append(e_sb)
                                    if len(fulls) == 2:
                                        es = p2.tile(
                                            [P, QBS], BF16,
                                            name=f"ep_{b}_{h}_{qb}_{kt}",
                                            tag="ep", bufs=3)
                                        nc.vector.tensor_add(
                                            es, fulls[0], fulls[1])
                                        fulls = []
                                        pairs.append(es)
                                        if len(pairs) == 2:
                                            eq = p2.tile(
                                                [P, QBS], BF16,
                                                name=f"eq_{b}_{h}_{qb}_{kt}",
                                                tag="eq", bufs=2)
                                            nc.vector.tensor_add(
                                                eq, pairs[0], pairs[1])
                                            pairs = []
                                            quads.append((kt + 3, eq))
                                if len(av_fifo) >= 3:
                                    emit_av(*av_fifo.pop(0))
                                av_fifo.append((kt, e_sb, lo))
                                if quads and kt >= quads[0][0]:
                                    emit_d(quads.pop(0)[1], 0, False)
                                if pend:
                                    emit_otile(*pend.pop(0))
                            # close the denominator group and start the
                            # normalization chain before draining the AV
                            # fifo, so 1/D is ready when u_ps closes
                            for _, eq in quads:
                                emit_d(eq, 0, False)
                            for dk in range(4):
                                emit_d(diag_e[dk], dk * P, dk == 3)
                            rf_sb = p2.tile([1, QBS], F32,
                                            name=f"rf_{b}_{h}_{qb}",
                                            tag="rf", bufs=2)
                            nc.vector.reciprocal_approx_fast(rf_sb, d_ps)
                            rb_sb = p2.tile([P, QBS], F32,
                                            name=f"rs_{b}_{h}_{qb}",
                                            tag="rs", bufs=2)
                            nc.gpsimd.partition_broadcast(rb_sb, rf_sb)
                            for a0 in av_fifo:
                                emit_av(*a0)
                            nc.vector.tensor_mul(
                                attn_sb[b][h][:, qb * QBS:(qb + 1) * QBS],
                                u_ps, rb_sb)

                        # queue this q-block's output tiles (heads done)
                        for i in range(4):
                            for ob2 in range(NOB // 2):
                                pend.append((b, qb * 4 + i, ob2))
                # drain whatever o-proj work is still queued
                for args in pend:
                    emit_otile(*args)

    nc.compile()
    return nc


def _prep_inputs(hidden_states, Wq, Wk, Wv, Wo, cos, sin):
    hs = np.asarray(hidden_states, dtype=np.float32)
    Wq = np.asarray(Wq, dtype=np.float32)
    Wk = np.asarray(Wk, dtype=np.float32)
    Wv = np.asarray(Wv, dtype=np.float32)
    Wo = np.asarray(Wo, dtype=np.float32)
    cos = np.asarray(cos, dtype=np.float32)
    sin = np.asarray(sin, dtype=np.float32)

    # x: [H, T] -> [NTB*P, NA*TB] so each (tb, 8-slice) load is one DMA
    # with 8KB contiguous per-partition runs
    xtm = np.ascontiguousarray(
        hs.reshape(T, H).T.reshape(NA, P, NTB, TB).transpose(2, 1, 0, 3)
        .reshape(NTB * P, NA * TB)).astype(NPBF)
    cosT = np.ascontiguousarray(cos.T).astype(NPBF)
    sinT = np.ascontiguousarray(sin.T)
    sints = np.ascontiguousarray(
        np.concatenate([-sinT[:64], sinT[64:]], axis=0)).astype(NPBF)
    kq = np.arange(P)
    trim = (kq[None, :] >= kq[:, None]).astype(NPBF)
    ident = np.eye(P, dtype=NPBF)
    onesc = np.ones((P, 1), dtype=NPBF)

    def pmajor(w):
        # [rows, cols] -> [128, (rows/128)*cols] partition-major layout
        r, ccols = w.shape
        return np.ascontiguousarray(
            w.reshape(r // P, P, ccols).transpose(1, 0, 2).reshape(P, -1)
        ).astype(NPBF)

    in_maps = []
    for c in range(8):
        in_maps.append({
            "xt": xtm,
            "wqt": pmajor(np.ascontiguousarray(Wq[c * FL:(c + 1) * FL, :].T)),
            "wkt": pmajor(np.ascontiguousarray(Wk[c * DK:(c + 1) * DK, :].T)),
            "wvt": pmajor(np.ascontiguousarray(Wv[c * DK:(c + 1) * DK, :].T)),
            "wot": pmajor(np.ascontiguousarray(Wo[:, c * FL:(c + 1) * FL].T)),
            "cost": cosT,
            "sints": sints,
            "trimask": trim,
            "identb": ident,
            "onesc": onesc,
        })
    return in_maps


def kernel(hidden_states, Wq, Wk, Wv, Wo, cos, sin, _run_kwargs=None):
    in_maps = _prep_inputs(hidden_states, Wq, Wk, Wv, Wo, cos, sin)
    if "nc" not in _NC_CACHE:
        _NC_CACHE["nc"] = build()
    nc = _NC_CACHE["nc"]
    kw = _run_kwargs or {}
    res = run_bass_kernel_spmd(nc, in_maps, core_ids=list(range(8)), **kw)
    acc = np.zeros((T, H), dtype=np.float64)
    for c in range(8):
        acc += np.asarray(res.results[c]["out"], dtype=np.float64)
    out = acc.astype(np.float32).reshape(B, S, H)
    if kw:
        _NC_CACHE["last_results"] = res
    return out
